# revision 5
# baseline (speedup 1.0000x reference)
"""Trainium2 Bass kernel: 16-head MHA (B=2, T=2048, D=1024, d_k=64).

Sharding (8 NeuronCores): data-parallel over the batch (2) x tensor-parallel
over head groups (4 groups of 4 heads).  Core c handles batch b = c//4 and
heads [4g, 4g+4) with g = c%4.  Each core computes its partial output
    sum_{h in group} softmax((q Wq_h + bq_h)(k Wk_h)^T / 8) (v Wv_h) Wo_h
and the host sums the 4 partials per batch and adds the constant row
bo + bv @ Wo once.  bk is dropped: with the all-ones mask it shifts every
score row by a per-row constant, which softmax ignores exactly.

Per-core pipeline (all fp32):
  1. Q^T, K^T, V^T projections: X^T (host-pretransposed) streamed from HBM,
     contracted over the model dim on the PE; Q^T/K^T kept as two
     [128, 2048] head-pair tiles (head h on partitions (h%2)*64..).
  2. V^T transposed back on the PE into 16 [128, 260] "V_ext" tiles: per
     head 64 V columns plus a ones column (computes the softmax row sums
     for free during the attention*V matmul).
  3. Per (head, 1024-wide q slice): scores transposed S'[k, q] = K Q^T in
     2-bank PSUM tiles, exp on the scalar engine (scale=1/8 folded into the
     activation), then O^T = V_ext^T E accumulated over the 16 k tiles.
     Row 64 of the accumulator is the softmax denominator; 1/x is computed
     as exp(-ln(x)) (both functions live in one ACT table set), broadcast
     across partitions with a rank-1 PE matmul, and applied on the DVE.
  4. Output projection O^T.T @ Wo accumulated over the 4 heads, interleaved
     with attention of the next q slice.
"""

import functools
import os

import numpy as np

import concourse.bass as bass
import concourse.mybir as mybir
import concourse.tile as tile
from concourse import bacc
from concourse.bass_utils import run_bass_kernel_spmd
from concourse.masks import make_identity

F32 = mybir.dt.float32
AFT = mybir.ActivationFunctionType

D = 1024          # model dim
T = 2048          # sequence length
B = 2             # batch
HEADS = 16        # total heads
DK = 64           # head dim
NCORES = 8
GH = 4            # heads per core
GD = GH * DK      # 256 projection cols per core
NF = D // 128     # 8 contraction chunks
NKT = T // 128    # 16 k/t tiles
SCALE = 1.0 / np.sqrt(np.float32(DK))  # 1/8

# Results of the last run (for test harness introspection: exec_time_ns etc.)
LAST_RESULTS = None


@functools.lru_cache(maxsize=1)
def _build_program():
    nc = bacc.Bacc("TRN2", target_bir_lowering=False, debug=False,
                   num_devices=NCORES)

    xqT = nc.declare_dram_parameter("xqT", [D, T], F32, isOutput=False)
    xkT = nc.declare_dram_parameter("xkT", [D, T], F32, isOutput=False)
    xvT = nc.declare_dram_parameter("xvT", [D, T], F32, isOutput=False)
    wq = nc.declare_dram_parameter("wq", [128, NF * GD], F32, isOutput=False)
    wk = nc.declare_dram_parameter("wk", [128, NF * GD], F32, isOutput=False)
    wv = nc.declare_dram_parameter("wv", [128, NF * GD], F32, isOutput=False)
    wo = nc.declare_dram_parameter("wo", [GH, DK, D], F32, isOutput=False)
    bqv = nc.declare_dram_parameter("bqv", [128, 2], F32, isOutput=False)
    out = nc.declare_dram_parameter("out", [T, D], F32, isOutput=True)

    with tile.TileContext(nc) as tc:
        # ---- persistent pools -------------------------------------------
        with (
            tc.tile_pool(name="qk", bufs=4) as qk_pool,
            tc.tile_pool(name="vext", bufs=NKT) as vext_pool,
            tc.tile_pool(name="wop", bufs=GH) as wo_pool,
            tc.tile_pool(name="const", bufs=1) as const_pool,
        ):
            ident = const_pool.tile([128, 128], F32, tag="ident")
            make_identity(nc, ident[:])
            ones_sb = const_pool.tile([1, DK], F32, tag="ones")
            nc.gpsimd.memset(ones_sb[:], 1.0)
            bqv_sb = const_pool.tile([128, 2], F32, tag="bqv")
            nc.sync.dma_start(bqv_sb[:], bqv[:])

            QT = [qk_pool.tile([128, T], F32, tag="qk", name=f"qt{m}")
                  for m in range(2)]
            KT = [qk_pool.tile([128, T], F32, tag="qk", name=f"kt{m}")
                  for m in range(2)]
            VE = [vext_pool.tile([128, GH * (DK + 1)], F32, tag="vext",
                                 name=f"ve{i}") for i in range(NKT)]
            WO = [wo_pool.tile([DK, D], F32, tag="wop", name=f"wo{h}")
                  for h in range(GH)]

            # ---- phase A: projections -----------------------------------
            with (
                tc.tile_pool(name="wts", bufs=3) as w_pool,
                tc.tile_pool(name="xt", bufs=3) as xt_pool,
                tc.tile_pool(name="vt", bufs=2) as vt_pool,
                tc.tile_pool(name="psA", bufs=8,
                             space=bass.MemorySpace.PSUM) as psA,
            ):
                VT = [vt_pool.tile([128, T], F32, tag="vt", name=f"vt{m}")
                      for m in range(2)]

                def projection(w_dram, x_dram, drain):
                    w_sb = w_pool.tile([128, NF * GD], F32, tag="w")
                    nc.sync.dma_start(w_sb[:], w_dram[:])
                    ps = [psA.tile([128, 512], F32, tag="proj", name=f"pj{i}")
                          for i in range(8)]
                    for fc in range(NF):
                        xt = xt_pool.tile([128, T], F32, tag="xt")
                        nc.sync.dma_start(
                            xt[:], x_dram[fc * 128:(fc + 1) * 128, :])
                        for m in range(2):
                            for qs in range(4):
                                nc.tensor.matmul(
                                    ps[m * 4 + qs][:],
                                    w_sb[:, fc * GD + m * 128:
                                         fc * GD + (m + 1) * 128],
                                    xt[:, qs * 512:(qs + 1) * 512],
                                    start=(fc == 0), stop=(fc == NF - 1))
                    for m in range(2):
                        for qs in range(4):
                            drain(m, qs, ps[m * 4 + qs])

                def q_drain(m, qs, ps):
                    nc.vector.tensor_scalar_add(
                        QT[m][:, qs * 512:(qs + 1) * 512], ps[:],
                        bqv_sb[:, m:m + 1])

                def k_drain(m, qs, ps):
                    nc.vector.tensor_copy(
                        KT[m][:, qs * 512:(qs + 1) * 512], ps[:])

                def v_drain(m, qs, ps):
                    nc.vector.tensor_copy(
                        VT[m][:, qs * 512:(qs + 1) * 512], ps[:])

                projection(wq, xqT, q_drain)
                projection(wk, xkT, k_drain)
                projection(wv, xvT, v_drain)

                # V^T -> V_ext (PE transpose of 128x128 blocks, per pair)
                for kt in range(NKT):
                    ve = VE[kt]
                    nc.gpsimd.memset(ve[:], 1.0)
                    ve_r = ve[:].rearrange("p (h x) -> p h x", x=DK + 1)
                    for m in range(2):
                        tp = psA.tile([128, 128], F32, tag="proj")
                        nc.tensor.transpose(
                            tp[:], VT[m][:, kt * 128:(kt + 1) * 128],
                            ident[:])
                        nc.vector.tensor_copy(
                            ve_r[:, 2 * m:2 * m + 2, 0:DK],
                            tp[:].rearrange("k (h j) -> k h j", j=DK))

            # ---- phase B: attention + output projection ------------------
            for h in range(GH):
                nc.sync.dma_start(WO[h][:], wo[h])

            with (
                tc.tile_pool(name="ep", bufs=8) as epool,
                tc.tile_pool(name="otp", bufs=GH) as ot_pool,
                tc.tile_pool(name="recp", bufs=4) as rec_pool,
                tc.tile_pool(name="rsbp", bufs=2) as rsb_pool,
                tc.tile_pool(name="osbp", bufs=3) as out_pool,
                tc.tile_pool(name="psS", bufs=2,
                             space=bass.MemorySpace.PSUM) as psS,
                tc.tile_pool(name="psO", bufs=2,
                             space=bass.MemorySpace.PSUM) as psO,
                tc.tile_pool(name="psR", bufs=1,
                             space=bass.MemorySpace.PSUM) as psR,
                tc.tile_pool(name="psF", bufs=1,
                             space=bass.MemorySpace.PSUM) as psF,
            ):
                OT = [ot_pool.tile([DK, T], F32, tag="ot", name=f"ot{h}")
                      for h in range(GH)]

                for qs in range(2):          # 1024-wide q slices
                    q0 = qs * 1024
                    for h in range(GH):
                        m, lo = h // 2, (h % 2) * DK
                        o_ps = [psO.tile([128, 512], F32, tag="o", name=f"o{i}")
                                for i in range(2)]
                        for kt in range(NKT):
                            s = psS.tile([128, 1024], F32, tag="s")
                            for hf in range(2):
                                nc.tensor.matmul(
                                    s[:, hf * 512:(hf + 1) * 512],
                                    KT[m][lo:lo + DK,
                                          kt * 128:(kt + 1) * 128],
                                    QT[m][lo:lo + DK,
                                          q0 + hf * 512:q0 + (hf + 1) * 512],
                                    start=True, stop=True)
                            e = epool.tile([128, 1024], F32, tag="e")
                            nc.scalar.activation(e[:], s[:], AFT.Exp,
                                                 scale=float(SCALE))
                            for hf in range(2):
                                nc.tensor.matmul(
                                    o_ps[hf][0:DK + 1, :],
                                    VE[kt][:, h * (DK + 1):
                                           (h + 1) * (DK + 1)],
                                    e[:, hf * 512:(hf + 1) * 512],
                                    start=(kt == 0), stop=(kt == NKT - 1))
                        for hf in range(2):
                            ln_t = rec_pool.tile([1, 512], F32, tag="rec")
                            rec_t = rec_pool.tile([1, 512], F32, tag="rec")
                            nc.scalar.activation(
                                ln_t[:], o_ps[hf][DK:DK + 1, :], AFT.Ln)
                            nc.scalar.activation(
                                rec_t[:], ln_t[:], AFT.Exp, scale=-1.0)
                            r_ps = psR.tile([128, 512], F32, tag="r")
                            nc.tensor.matmul(
                                r_ps[0:DK, :], ones_sb[:], rec_t[:],
                                start=True, stop=True)
                            r_sb = rsb_pool.tile([DK, 512], F32, tag="rsb")
                            nc.vector.tensor_copy(r_sb[:], r_ps[0:DK, :])
                            nc.vector.tensor_mul(
                                OT[h][:, q0 + hf * 512:q0 + (hf + 1) * 512],
                                o_ps[hf][0:DK, :], r_sb[:])

                    # output projection for the 8 t-tiles of this q slice
                    for tt in range(qs * 8, (qs + 1) * 8):
                        osb = out_pool.tile([128, 1024], F32, tag="osb")
                        for ei in range(2):
                            f_ps = psF.tile([128, 512], F32, tag="f")
                            for h in range(GH):
                                nc.tensor.matmul(
                                    f_ps[:],
                                    OT[h][:, tt * 128:(tt + 1) * 128],
                                    WO[h][:, ei * 512:(ei + 1) * 512],
                                    start=(h == 0), stop=(h == GH - 1))
                            nc.vector.tensor_copy(
                                osb[:, ei * 512:(ei + 1) * 512], f_ps[:])
                        nc.sync.dma_start(
                            out[tt * 128:(tt + 1) * 128, :], osb[:])

    nc.compile()
    return nc


def _numpy_reference(q, k, v, mask, Wq, bq, Wk, bk, Wv, bv, Wo, bo):
    """Fallback for a non-trivial mask (never hit with the stock inputs)."""
    Bn, Tn, _ = q.shape
    H, dk = HEADS, DK

    def split(x):
        return x.reshape(Bn, Tn, H, dk).transpose(0, 2, 1, 3)

    qh = split(q @ Wq + bq)
    kh = split(k @ Wk + bk)
    vh = split(v @ Wv + bv)
    s = np.einsum("bhqd,bhkd->bhqk", qh, kh) / np.sqrt(np.float32(dk))
    s = np.where(mask, s, -np.inf)
    s = s - s.max(axis=-1, keepdims=True)
    e = np.exp(s)
    a = e / e.sum(axis=-1, keepdims=True)
    o = np.einsum("bhqk,bhkd->bhqd", a, vh)
    o = o.transpose(0, 2, 1, 3).reshape(Bn, Tn, H * dk)
    return (o @ Wo + bo).astype(np.float32)


def kernel(q, k, v, mask, Wq, bq, Wk, bk, Wv, bv, Wo, bo):
    global LAST_RESULTS
    q = np.asarray(q, np.float32)
    k = np.asarray(k, np.float32)
    v = np.asarray(v, np.float32)
    mask = np.asarray(mask, bool)
    Wq, bq = np.asarray(Wq, np.float32), np.asarray(bq, np.float32)
    Wk, bk = np.asarray(Wk, np.float32), np.asarray(bk, np.float32)
    Wv, bv = np.asarray(Wv, np.float32), np.asarray(bv, np.float32)
    Wo, bo = np.asarray(Wo, np.float32), np.asarray(bo, np.float32)

    if not mask.all():
        return _numpy_reference(q, k, v, mask, Wq, bq, Wk, bk, Wv, bv, Wo, bo)

    nc = _build_program()

    # host-side sharding
    xT = {}
    for b in range(B):
        xT[b] = tuple(np.ascontiguousarray(x[b].T) for x in (q, k, v))

    def w_chunks(W, g):
        # (1024, 256) head-group slice -> [128, 8*256] chunk-major layout
        Wg = W[:, g * GD:(g + 1) * GD]
        return np.ascontiguousarray(
            Wg.reshape(NF, 128, GD).transpose(1, 0, 2).reshape(128, NF * GD))

    in_maps = []
    for c in range(NCORES):
        b, g = divmod(c, GH)
        xq_t, xk_t, xv_t = xT[b]
        in_maps.append({
            "xqT": xq_t, "xkT": xk_t, "xvT": xv_t,
            "wq": w_chunks(Wq, g), "wk": w_chunks(Wk, g),
            "wv": w_chunks(Wv, g),
            "wo": np.ascontiguousarray(
                Wo[g * GD:(g + 1) * GD, :]).reshape(GH, DK, D),
            "bqv": np.ascontiguousarray(
                bq[g * GD:(g + 1) * GD].reshape(2, 128).T),
        })

    LAST_RESULTS = run_bass_kernel_spmd(
        nc, in_maps, list(range(NCORES)),
        trace=bool(os.environ.get("KERNEL_TRACE")))
    res = LAST_RESULTS.results

    const_row = (bv @ Wo + bo).astype(np.float32)  # attn rows sum to 1
    full = np.empty((B, T, D), np.float32)
    for b in range(B):
        acc = res[b * GH]["out"].astype(np.float32)
        for g in range(1, GH):
            acc = acc + res[b * GH + g]["out"]
        full[b] = acc + const_row
    return full


# revision 11
# speedup vs baseline: 2.0891x; 2.0891x over previous
"""Trainium2 Bass kernel: 16-head MHA (B=2, T=2048, D=1024, d_k=64).

Sharding (8 NeuronCores): data-parallel over the batch (2) x tensor-parallel
over head groups (4 groups of 4 heads).  Core c handles batch b = c//4 and
heads [4g, 4g+4) with g = c%4.  Each core computes its partial output
    sum_{h in group} softmax((q Wq_h + bq_h)(k Wk_h)^T / 8) (v Wv_h) Wo_h
and the host sums the 4 partials per batch and adds the constant row
bo + bv @ Wo once.  bk is dropped: with the all-ones mask it shifts every
score row by a per-row constant, which softmax ignores exactly.

Per-core pipeline (all fp32):
  1. Q^T, K^T, V^T projections: X^T (host-pretransposed) streamed from HBM,
     contracted over the model dim on the PE; Q^T/K^T kept as two
     [128, 2048] head-pair tiles (head h on partitions (h%2)*64..).
  2. V^T transposed back on the PE into 16 [128, 260] "V_ext" tiles: per
     head 64 V columns plus a ones column (computes the softmax row sums
     for free during the attention*V matmul).
  3. Per (head, 1024-wide q slice): scores transposed S'[k, q] = K Q^T in
     2-bank PSUM tiles, exp on the scalar engine (scale=1/8 folded into the
     activation), then O^T = V_ext^T E accumulated over the 16 k tiles.
     Row 64 of the accumulator is the softmax denominator; 1/x is computed
     as exp(-ln(x)) (both functions live in one ACT table set), broadcast
     across partitions with a rank-1 PE matmul, and applied on the DVE.
  4. Output projection O^T.T @ Wo accumulated over the 4 heads, interleaved
     with attention of the next q slice.
"""

import functools
import os

import numpy as np

import concourse.bass as bass
import concourse.mybir as mybir
import concourse.tile as tile
from concourse import bacc
from concourse.bass_utils import run_bass_kernel_spmd
from concourse.masks import make_identity

F32 = mybir.dt.float32
F32R = mybir.dt.float32r   # single-pass PE fp32 (4x faster at N>=256)
AFT = mybir.ActivationFunctionType


D = 1024          # model dim
T = 2048          # sequence length
B = 2             # batch
HEADS = 16        # total heads
DK = 64           # head dim
NCORES = 8
GH = 4            # heads per core
GD = GH * DK      # 256 projection cols per core
NF = D // 128     # 8 contraction chunks
NKT = T // 128    # 16 k/t tiles
SCALE = 1.0 / np.sqrt(np.float32(DK))  # 1/8

# Results of the last run (for test harness introspection: exec_time_ns etc.)
LAST_RESULTS = None


@functools.lru_cache(maxsize=1)
def _build_program():
    nc = bacc.Bacc("TRN2", target_bir_lowering=False, debug=False,
                   num_devices=NCORES)

    xqT = nc.declare_dram_parameter("xqT", [D, T], F32R, isOutput=False)
    xkT = nc.declare_dram_parameter("xkT", [D, T], F32R, isOutput=False)
    xvT = nc.declare_dram_parameter("xvT", [D, T], F32R, isOutput=False)
    wq = nc.declare_dram_parameter("wq", [128, NF * GD], F32R, isOutput=False)
    wk = nc.declare_dram_parameter("wk", [128, NF * GD], F32R, isOutput=False)
    wv = nc.declare_dram_parameter("wv", [128, NF * GD], F32R, isOutput=False)
    wo = nc.declare_dram_parameter("wo", [GH, DK, D], F32R, isOutput=False)
    bqv = nc.declare_dram_parameter("bqv", [128, 2], F32, isOutput=False)
    out = nc.declare_dram_parameter("out", [T, D], F32, isOutput=True)

    with tile.TileContext(nc) as tc:
        # ---- persistent pools -------------------------------------------
        with (
            tc.tile_pool(name="qk", bufs=4) as qk_pool,
            tc.tile_pool(name="vext", bufs=NKT) as vext_pool,
            tc.tile_pool(name="wop", bufs=GH) as wo_pool,
            tc.tile_pool(name="const", bufs=1) as const_pool,
        ):
            ident = const_pool.tile([128, 128], F32, tag="ident")
            make_identity(nc, ident[:])
            ones_f32 = const_pool.tile([128, DK], F32, tag="ones32")
            nc.gpsimd.memset(ones_f32[:], 1.0)
            ones_sb = const_pool.tile([1, DK], F32R, tag="ones")
            nc.vector.tensor_copy(ones_sb[:], ones_f32[0:1, :])
            bqv_sb = const_pool.tile([128, 2], F32, tag="bqv")
            nc.sync.dma_start(bqv_sb[:], bqv[:])

            QT = [qk_pool.tile([128, T], F32R, tag="qk", name=f"qt{m}")
                  for m in range(2)]
            KT = [qk_pool.tile([128, T], F32R, tag="qk", name=f"kt{m}")
                  for m in range(2)]
            VE = [vext_pool.tile([128, GH * (DK + 1)], F32R, tag="vext",
                                 name=f"ve{i}") for i in range(NKT)]
            WO = [wo_pool.tile([DK, D], F32R, tag="wop", name=f"wo{h}")
                  for h in range(GH)]

            # ---- phase A: projections -----------------------------------
            with (
                tc.tile_pool(name="wts", bufs=3) as w_pool,
                tc.tile_pool(name="xt", bufs=3) as xt_pool,
                tc.tile_pool(name="vt", bufs=2) as vt_pool,
                tc.tile_pool(name="psA", bufs=8,
                             space=bass.MemorySpace.PSUM) as psA,
            ):
                VT = [vt_pool.tile([128, T], F32, tag="vt", name=f"vt{m}")
                      for m in range(2)]

                def projection(w_dram, x_dram, drain):
                    w_sb = w_pool.tile([128, NF * GD], F32R, tag="w")
                    nc.sync.dma_start(w_sb[:], w_dram[:])
                    ps = [psA.tile([128, 512], F32, tag="proj", name=f"pj{i}")
                          for i in range(8)]
                    for fc in range(NF):
                        xt = xt_pool.tile([128, T], F32R, tag="xt")
                        nc.sync.dma_start(
                            xt[:], x_dram[fc * 128:(fc + 1) * 128, :])
                        for m in range(2):
                            for qs in range(4):
                                nc.tensor.matmul(
                                    ps[m * 4 + qs][:],
                                    (w_sb[:, fc * GD + m * 128:
                                            fc * GD + (m + 1) * 128]),
                                    (xt[:, qs * 512:(qs + 1) * 512]),
                                    start=(fc == 0), stop=(fc == NF - 1))
                    for m in range(2):
                        for qs in range(4):
                            drain(m, qs, ps[m * 4 + qs])

                def q_drain(m, qs, ps):
                    nc.vector.tensor_scalar_add(
                        QT[m][:, qs * 512:(qs + 1) * 512], ps[:],
                        bqv_sb[:, m:m + 1])

                def k_drain(m, qs, ps):
                    nc.vector.tensor_copy(
                        KT[m][:, qs * 512:(qs + 1) * 512], ps[:])

                def v_drain(m, qs, ps):
                    nc.vector.tensor_copy(
                        VT[m][:, qs * 512:(qs + 1) * 512], ps[:])

                projection(wq, xqT, q_drain)
                projection(wk, xkT, k_drain)
                projection(wv, xvT, v_drain)

                # V^T -> V_ext (PE transpose of 128x128 blocks, per pair)
                for kt in range(NKT):
                    ve = VE[kt]
                    ve_r = ve[:].rearrange("p (h x) -> p h x", x=DK + 1)
                    nc.vector.tensor_copy(
                        ve_r[:, :, DK:DK + 1],
                        ones_f32[:, 0:GH].rearrange("p (h x) -> p h x", x=1))
                    for m in range(2):
                        tp = psA.tile([128, 128], F32, tag="proj")
                        nc.tensor.transpose(
                            tp[:], VT[m][:, kt * 128:(kt + 1) * 128],
                            ident[:])
                        nc.vector.tensor_copy(
                            ve_r[:, 2 * m:2 * m + 2, 0:DK],
                            tp[:].rearrange("k (h j) -> k h j", j=DK))

            # ---- phase B: attention + output projection ------------------
            for h in range(GH):
                nc.sync.dma_start(WO[h][:], wo[h])

            with (
                tc.tile_pool(name="ep", bufs=6) as epool,
                tc.tile_pool(name="otp", bufs=GH) as ot_pool,
                tc.tile_pool(name="ubp", bufs=8) as ub_pool,
                tc.tile_pool(name="rsp", bufs=2) as rs_pool,
                tc.tile_pool(name="rsbp", bufs=2) as rsb_pool,
                tc.tile_pool(name="osbp", bufs=3) as out_pool,
                tc.tile_pool(name="psS", bufs=2,
                             space=bass.MemorySpace.PSUM) as psS,
                tc.tile_pool(name="psO", bufs=2,
                             space=bass.MemorySpace.PSUM) as psO,
                tc.tile_pool(name="psR", bufs=1,
                             space=bass.MemorySpace.PSUM) as psR,
                tc.tile_pool(name="psF", bufs=1,
                             space=bass.MemorySpace.PSUM) as psF,
            ):
                OT = [ot_pool.tile([DK, T], F32R, tag="ot", name=f"ot{h}")
                      for h in range(GH)]

                for qs in range(2):          # 1024-wide q slices
                    q0 = qs * 1024
                    # rowsums of all 8 (head, half) pairs of this group,
                    # collected on one partition for a single batched 1/x
                    rs = rs_pool.tile([1, 8 * 512], F32R, tag="rs")
                    ub = {}
                    for h in range(GH):
                        m, lo = h // 2, (h % 2) * DK
                        o_ps = [psO.tile([128, 512], F32, tag="o", name=f"o{i}")
                                for i in range(2)]
                        for kt in range(NKT):
                            s = psS.tile([128, 1024], F32, tag="s")
                            for hf in range(2):
                                nc.tensor.matmul(
                                    s[:, hf * 512:(hf + 1) * 512],
                                    (KT[m][lo:lo + DK,
                                             kt * 128:(kt + 1) * 128]),
                                    (QT[m][lo:lo + DK,
                                             q0 + hf * 512:
                                             q0 + (hf + 1) * 512]),
                                    start=True, stop=True)
                            e = epool.tile([128, 1024], F32R, tag="e")
                            nc.scalar.activation(e[:], s[:], AFT.Exp,
                                                 scale=float(SCALE))
                            for hf in range(2):
                                nc.tensor.matmul(
                                    o_ps[hf][0:DK + 1, :],
                                    (VE[kt][:, h * (DK + 1):
                                              (h + 1) * (DK + 1)]),
                                    (e[:, hf * 512:(hf + 1) * 512]),
                                    start=(kt == 0), stop=(kt == NKT - 1))
                        for hf in range(2):
                            i8 = h * 2 + hf
                            u = ub_pool.tile([DK, 512], F32, tag="ub",
                                             name=f"ub{i8}")
                            nc.vector.tensor_copy(u[:], o_ps[hf][0:DK, :])
                            nc.vector.tensor_copy(
                                rs[0:1, i8 * 512:(i8 + 1) * 512],
                                o_ps[hf][DK:DK + 1, :])
                            ub[i8] = u

                    # one batched reciprocal for the whole group: 1/x via
                    # exp(-ln(x)), in place on the collected rowsums
                    nc.scalar.activation(rs[:], rs[:], AFT.Ln)
                    nc.scalar.activation(rs[:], rs[:], AFT.Exp, scale=-1.0)

                    for i8 in range(8):
                        h, hf = divmod(i8, 2)
                        r_ps = psR.tile([128, 512], F32, tag="r")
                        nc.tensor.matmul(
                            r_ps[0:DK, :], (ones_sb[:]),
                            (rs[0:1, i8 * 512:(i8 + 1) * 512]),
                            start=True, stop=True)
                        r_sb = rsb_pool.tile([DK, 512], F32, tag="rsb")
                        nc.vector.tensor_copy(r_sb[:], r_ps[0:DK, :])
                        nc.vector.tensor_mul(
                            OT[h][:, q0 + hf * 512:q0 + (hf + 1) * 512],
                            ub[i8][:], r_sb[:])

                    # output projection for the 8 t-tiles of this q slice
                    for tt in range(qs * 8, (qs + 1) * 8):
                        osb = out_pool.tile([128, 1024], F32, tag="osb")
                        for ei in range(2):
                            f_ps = psF.tile([128, 512], F32, tag="f")
                            for h in range(GH):
                                nc.tensor.matmul(
                                    f_ps[:],
                                    (OT[h][:, tt * 128:(tt + 1) * 128]),
                                    (WO[h][:, ei * 512:(ei + 1) * 512]),
                                    start=(h == 0), stop=(h == GH - 1))
                            nc.vector.tensor_copy(
                                osb[:, ei * 512:(ei + 1) * 512], f_ps[:])
                        nc.sync.dma_start(
                            out[tt * 128:(tt + 1) * 128, :], osb[:])

    from concourse.bacc import get_activation_tables
    import bass_rust as _br
    _combined = "natural_log_exp_and_others"
    _tabs = []
    for _name, _fns in get_activation_tables(nc.m.arch).items():
        if _name != _combined:
            _fns = _fns - {AFT.Exp, AFT.Ln}
        _tabs.append((_name, _fns))
    _br.insert_act_table_loads(nc, _tabs)
    nc.compile()
    return nc


def _numpy_reference(q, k, v, mask, Wq, bq, Wk, bk, Wv, bv, Wo, bo):
    """Fallback for a non-trivial mask (never hit with the stock inputs)."""
    Bn, Tn, _ = q.shape
    H, dk = HEADS, DK

    def split(x):
        return x.reshape(Bn, Tn, H, dk).transpose(0, 2, 1, 3)

    qh = split(q @ Wq + bq)
    kh = split(k @ Wk + bk)
    vh = split(v @ Wv + bv)
    s = np.einsum("bhqd,bhkd->bhqk", qh, kh) / np.sqrt(np.float32(dk))
    s = np.where(mask, s, -np.inf)
    s = s - s.max(axis=-1, keepdims=True)
    e = np.exp(s)
    a = e / e.sum(axis=-1, keepdims=True)
    o = np.einsum("bhqk,bhkd->bhqd", a, vh)
    o = o.transpose(0, 2, 1, 3).reshape(Bn, Tn, H * dk)
    return (o @ Wo + bo).astype(np.float32)


def kernel(q, k, v, mask, Wq, bq, Wk, bk, Wv, bv, Wo, bo):
    global LAST_RESULTS
    q = np.asarray(q, np.float32)
    k = np.asarray(k, np.float32)
    v = np.asarray(v, np.float32)
    mask = np.asarray(mask, bool)
    Wq, bq = np.asarray(Wq, np.float32), np.asarray(bq, np.float32)
    Wk, bk = np.asarray(Wk, np.float32), np.asarray(bk, np.float32)
    Wv, bv = np.asarray(Wv, np.float32), np.asarray(bv, np.float32)
    Wo, bo = np.asarray(Wo, np.float32), np.asarray(bo, np.float32)

    if not mask.all():
        return _numpy_reference(q, k, v, mask, Wq, bq, Wk, bk, Wv, bv, Wo, bo)

    nc = _build_program()

    # host-side sharding
    xT = {}
    for b in range(B):
        xT[b] = tuple(np.ascontiguousarray(x[b].T) for x in (q, k, v))

    def w_chunks(W, g):
        # (1024, 256) head-group slice -> [128, 8*256] chunk-major layout
        Wg = W[:, g * GD:(g + 1) * GD]
        return np.ascontiguousarray(
            Wg.reshape(NF, 128, GD).transpose(1, 0, 2).reshape(128, NF * GD))

    in_maps = []
    for c in range(NCORES):
        b, g = divmod(c, GH)
        xq_t, xk_t, xv_t = xT[b]
        in_maps.append({
            "xqT": xq_t, "xkT": xk_t, "xvT": xv_t,
            "wq": w_chunks(Wq, g), "wk": w_chunks(Wk, g),
            "wv": w_chunks(Wv, g),
            "wo": np.ascontiguousarray(
                Wo[g * GD:(g + 1) * GD, :]).reshape(GH, DK, D),
            "bqv": np.ascontiguousarray(
                bq[g * GD:(g + 1) * GD].reshape(2, 128).T),
        })

    LAST_RESULTS = run_bass_kernel_spmd(
        nc, in_maps, list(range(NCORES)),
        trace=bool(os.environ.get("KERNEL_TRACE")))
    res = LAST_RESULTS.results

    const_row = (bv @ Wo + bo).astype(np.float32)  # attn rows sum to 1
    full = np.empty((B, T, D), np.float32)
    for b in range(B):
        acc = res[b * GH]["out"].astype(np.float32)
        for g in range(1, GH):
            acc = acc + res[b * GH + g]["out"]
        full[b] = acc + const_row
    return full


# revision 12
# speedup vs baseline: 2.1484x; 1.0284x over previous
"""Trainium2 Bass kernel: 16-head MHA (B=2, T=2048, D=1024, d_k=64).

Sharding (8 NeuronCores): data-parallel over the batch (2) x tensor-parallel
over head groups (4 groups of 4 heads).  Core c handles batch b = c//4 and
heads [4g, 4g+4) with g = c%4.  Each core computes its partial output
    sum_{h in group} softmax((q Wq_h + bq_h)(k Wk_h)^T / 8) (v Wv_h) Wo_h
and the host sums the 4 partials per batch and adds the constant row
bo + bv @ Wo once.  bk is dropped: with the all-ones mask it shifts every
score row by a per-row constant, which softmax ignores exactly.

Per-core pipeline (all fp32):
  1. Q^T, K^T, V^T projections: X^T (host-pretransposed) streamed from HBM,
     contracted over the model dim on the PE; Q^T/K^T kept as two
     [128, 2048] head-pair tiles (head h on partitions (h%2)*64..).
  2. V^T transposed back on the PE into 16 [128, 260] "V_ext" tiles: per
     head 64 V columns plus a ones column (computes the softmax row sums
     for free during the attention*V matmul).
  3. Per (head, 1024-wide q slice): scores transposed S'[k, q] = K Q^T in
     2-bank PSUM tiles, exp on the scalar engine (scale=1/8 folded into the
     activation), then O^T = V_ext^T E accumulated over the 16 k tiles.
     Row 64 of the accumulator is the softmax denominator; 1/x is computed
     as exp(-ln(x)) (both functions live in one ACT table set), broadcast
     across partitions with a rank-1 PE matmul, and applied on the DVE.
  4. Output projection O^T.T @ Wo accumulated over the 4 heads, interleaved
     with attention of the next q slice.
"""

import functools
import os

import numpy as np

import concourse.bass as bass
import concourse.mybir as mybir
import concourse.tile as tile
from concourse import bacc
from concourse.bass_utils import run_bass_kernel_spmd
from concourse.masks import make_identity

F32 = mybir.dt.float32
F32R = mybir.dt.float32r   # single-pass PE fp32 (4x faster at N>=256)
AFT = mybir.ActivationFunctionType


D = 1024          # model dim
T = 2048          # sequence length
B = 2             # batch
HEADS = 16        # total heads
DK = 64           # head dim
NCORES = 8
GH = 4            # heads per core
GD = GH * DK      # 256 projection cols per core
NF = D // 128     # 8 contraction chunks
NKT = T // 128    # 16 k/t tiles
SCALE = 1.0 / np.sqrt(np.float32(DK))  # 1/8

# Results of the last run (for test harness introspection: exec_time_ns etc.)
LAST_RESULTS = None


@functools.lru_cache(maxsize=1)
def _build_program():
    nc = bacc.Bacc("TRN2", target_bir_lowering=False, debug=False,
                   num_devices=NCORES)

    xqT = nc.declare_dram_parameter("xqT", [D, T], F32R, isOutput=False)
    xkT = nc.declare_dram_parameter("xkT", [D, T], F32R, isOutput=False)
    xvT = nc.declare_dram_parameter("xvT", [D, T], F32R, isOutput=False)
    wq = nc.declare_dram_parameter("wq", [128, NF * GD], F32R, isOutput=False)
    wk = nc.declare_dram_parameter("wk", [128, NF * GD], F32R, isOutput=False)
    wv = nc.declare_dram_parameter("wv", [128, NF * GD], F32R, isOutput=False)
    wo = nc.declare_dram_parameter("wo", [GH, DK, D], F32R, isOutput=False)
    bqv = nc.declare_dram_parameter("bqv", [128, 2], F32, isOutput=False)
    out = nc.declare_dram_parameter("out", [T, D], F32, isOutput=True)

    with tile.TileContext(nc) as tc:
        # ---- persistent pools -------------------------------------------
        with (
            tc.tile_pool(name="qk", bufs=4) as qk_pool,
            tc.tile_pool(name="vext", bufs=NKT) as vext_pool,
            tc.tile_pool(name="wop", bufs=GH) as wo_pool,
            tc.tile_pool(name="const", bufs=1) as const_pool,
        ):
            ident = const_pool.tile([128, 128], F32, tag="ident")
            make_identity(nc, ident[:])
            ones_f32 = const_pool.tile([128, DK], F32, tag="ones32")
            nc.gpsimd.memset(ones_f32[:], 1.0)
            ones_sb = const_pool.tile([1, DK], F32R, tag="ones")
            nc.vector.tensor_copy(ones_sb[:], ones_f32[0:1, :])
            bqv_sb = const_pool.tile([128, 2], F32, tag="bqv")
            nc.sync.dma_start(bqv_sb[:], bqv[:])

            QT = [qk_pool.tile([128, T], F32R, tag="qk", name=f"qt{m}")
                  for m in range(2)]
            KT = [qk_pool.tile([128, T], F32R, tag="qk", name=f"kt{m}")
                  for m in range(2)]
            VE = [vext_pool.tile([128, GH * (DK + 1)], F32R, tag="vext",
                                 name=f"ve{i}") for i in range(NKT)]
            WO = [wo_pool.tile([DK, D], F32R, tag="wop", name=f"wo{h}")
                  for h in range(GH)]

            # ---- phase A: projections -----------------------------------
            with (
                tc.tile_pool(name="wts", bufs=3) as w_pool,
                tc.tile_pool(name="xt", bufs=3) as xt_pool,
                tc.tile_pool(name="vt", bufs=2) as vt_pool,
                tc.tile_pool(name="psA", bufs=8,
                             space=bass.MemorySpace.PSUM) as psA,
            ):
                VT = [vt_pool.tile([128, T], F32, tag="vt", name=f"vt{m}")
                      for m in range(2)]

                def projection(w_dram, x_dram, drain):
                    w_sb = w_pool.tile([128, NF * GD], F32R, tag="w")
                    nc.sync.dma_start(w_sb[:], w_dram[:])
                    ps = [psA.tile([128, 512], F32, tag="proj", name=f"pj{i}")
                          for i in range(8)]
                    for fc in range(NF):
                        xt = xt_pool.tile([128, T], F32R, tag="xt")
                        nc.sync.dma_start(
                            xt[:], x_dram[fc * 128:(fc + 1) * 128, :])
                        for m in range(2):
                            for qs in range(4):
                                nc.tensor.matmul(
                                    ps[m * 4 + qs][:],
                                    (w_sb[:, fc * GD + m * 128:
                                            fc * GD + (m + 1) * 128]),
                                    (xt[:, qs * 512:(qs + 1) * 512]),
                                    start=(fc == 0), stop=(fc == NF - 1))
                    for m in range(2):
                        for qs in range(4):
                            drain(m, qs, ps[m * 4 + qs])

                def q_drain(m, qs, ps):
                    nc.vector.tensor_scalar_add(
                        QT[m][:, qs * 512:(qs + 1) * 512], ps[:],
                        bqv_sb[:, m:m + 1])

                def k_drain(m, qs, ps):
                    nc.vector.tensor_copy(
                        KT[m][:, qs * 512:(qs + 1) * 512], ps[:])

                def v_drain(m, qs, ps):
                    nc.vector.tensor_copy(
                        VT[m][:, qs * 512:(qs + 1) * 512], ps[:])

                projection(wq, xqT, q_drain)
                projection(wk, xkT, k_drain)
                projection(wv, xvT, v_drain)

                # V^T -> V_ext (PE transpose of 128x128 blocks, per pair)
                for kt in range(NKT):
                    ve = VE[kt]
                    ve_r = ve[:].rearrange("p (h x) -> p h x", x=DK + 1)
                    nc.vector.tensor_copy(
                        ve_r[:, :, DK:DK + 1],
                        ones_f32[:, 0:GH].rearrange("p (h x) -> p h x", x=1))
                    for m in range(2):
                        tp = psA.tile([128, 128], F32, tag="proj")
                        nc.tensor.transpose(
                            tp[:], VT[m][:, kt * 128:(kt + 1) * 128],
                            ident[:])
                        nc.vector.tensor_copy(
                            ve_r[:, 2 * m:2 * m + 2, 0:DK],
                            tp[:].rearrange("k (h j) -> k h j", j=DK))

            # ---- phase B: attention + output projection ------------------
            for h in range(GH):
                nc.sync.dma_start(WO[h][:], wo[h])

            with (
                tc.tile_pool(name="ep", bufs=8) as epool,
                tc.tile_pool(name="otp", bufs=GH) as ot_pool,
                tc.tile_pool(name="ubp", bufs=8) as ub_pool,
                tc.tile_pool(name="rsp", bufs=1) as rs_pool,
                tc.tile_pool(name="rsbp", bufs=2) as rsb_pool,
                tc.tile_pool(name="psS", bufs=2,
                             space=bass.MemorySpace.PSUM) as psS,
                tc.tile_pool(name="psO", bufs=2,
                             space=bass.MemorySpace.PSUM) as psO,
            ):
                OT = [ot_pool.tile([DK, T], F32R, tag="ot", name=f"ot{h}")
                      for h in range(GH)]

                for qs in range(2):          # 1024-wide q slices
                    q0 = qs * 1024
                    # rowsums of all 8 (head, half) pairs of this group,
                    # collected on one partition for a single batched 1/x
                    rs = rs_pool.tile([1, 8 * 512], F32R, tag="rs")
                    ub = {}
                    for hp in range(2):      # head pairs -> PE row groups
                        o_ps = [psO.tile([128, 1024], F32, tag="o",
                                         name=f"o{i}") for i in range(2)]
                        for kt in range(NKT):
                            es = []
                            for hh in range(2):   # head within pair
                                lo = hh * DK
                                s = psS.tile([128, 1024], F32, tag="s")
                                for hf in range(2):
                                    nc.tensor.matmul(
                                        s[:, hf * 512:(hf + 1) * 512],
                                        (KT[hp][lo:lo + DK,
                                                kt * 128:(kt + 1) * 128]),
                                        (QT[hp][lo:lo + DK,
                                                q0 + hf * 512:
                                                q0 + (hf + 1) * 512]),
                                        start=True, stop=True)
                                e = epool.tile([128, 1024], F32R, tag="e")
                                nc.scalar.activation(e[:], s[:], AFT.Exp,
                                                     scale=float(SCALE))
                                es.append(e)
                            for hh in range(2):
                                h = hp * 2 + hh
                                for hf in range(2):
                                    nc.tensor.matmul(
                                        o_ps[hh][0:DK + 1,
                                                 hf * 512:(hf + 1) * 512],
                                        (VE[kt][:, h * (DK + 1):
                                                (h + 1) * (DK + 1)]),
                                        (es[hh][:, hf * 512:(hf + 1) * 512]),
                                        start=(kt == 0), stop=(kt == NKT - 1))
                        for hh in range(2):
                            h = hp * 2 + hh
                            for hf in range(2):
                                i8 = h * 2 + hf
                                u = ub_pool.tile([DK, 512], F32, tag="ub",
                                                 name=f"ub{i8}")
                                nc.vector.tensor_copy(
                                    u[:], o_ps[hh][0:DK,
                                                   hf * 512:(hf + 1) * 512])
                                nc.vector.tensor_copy(
                                    rs[0:1, i8 * 512:(i8 + 1) * 512],
                                    o_ps[hh][DK:DK + 1,
                                             hf * 512:(hf + 1) * 512])
                                ub[i8] = u

                    # one batched reciprocal for the whole group: 1/x via
                    # exp(-ln(x)), in place on the collected rowsums
                    nc.scalar.activation(rs[:], rs[:], AFT.Ln)
                    nc.scalar.activation(rs[:], rs[:], AFT.Exp, scale=-1.0)

                    for i8 in range(8):
                        h, hf = divmod(i8, 2)
                        r_ps = psS.tile([128, 1024], F32, tag="s",
                                        name=f"rps{i8}")
                        nc.tensor.matmul(
                            r_ps[0:DK, 0:512], (ones_sb[:]),
                            (rs[0:1, i8 * 512:(i8 + 1) * 512]),
                            start=True, stop=True)
                        r_sb = rsb_pool.tile([DK, 512], F32, tag="rsb")
                        nc.vector.tensor_copy(r_sb[:], r_ps[0:DK, 0:512])
                        nc.vector.tensor_mul(
                            OT[h][:, q0 + hf * 512:q0 + (hf + 1) * 512],
                            ub[i8][:], r_sb[:])

            # ---- output projection tail ---------------------------------
            with (
                tc.tile_pool(name="osbp", bufs=4) as out_pool,
                tc.tile_pool(name="psF", bufs=4,
                             space=bass.MemorySpace.PSUM) as psF,
            ):
                for tt in range(NKT):
                    osb = out_pool.tile([128, 1024], F32, tag="osb")
                    for ei in range(2):
                        f_ps = psF.tile([128, 512], F32, tag="f")
                        for h in range(GH):
                            nc.tensor.matmul(
                                f_ps[:],
                                (OT[h][:, tt * 128:(tt + 1) * 128]),
                                (WO[h][:, ei * 512:(ei + 1) * 512]),
                                start=(h == 0), stop=(h == GH - 1))
                        nc.vector.tensor_copy(
                            osb[:, ei * 512:(ei + 1) * 512], f_ps[:])
                    nc.sync.dma_start(
                        out[tt * 128:(tt + 1) * 128, :], osb[:])

    from concourse.bacc import get_activation_tables
    import bass_rust as _br
    _combined = "natural_log_exp_and_others"
    _tabs = []
    for _name, _fns in get_activation_tables(nc.m.arch).items():
        if _name != _combined:
            _fns = _fns - {AFT.Exp, AFT.Ln}
        _tabs.append((_name, _fns))
    _br.insert_act_table_loads(nc, _tabs)
    nc.compile()
    return nc


def _numpy_reference(q, k, v, mask, Wq, bq, Wk, bk, Wv, bv, Wo, bo):
    """Fallback for a non-trivial mask (never hit with the stock inputs)."""
    Bn, Tn, _ = q.shape
    H, dk = HEADS, DK

    def split(x):
        return x.reshape(Bn, Tn, H, dk).transpose(0, 2, 1, 3)

    qh = split(q @ Wq + bq)
    kh = split(k @ Wk + bk)
    vh = split(v @ Wv + bv)
    s = np.einsum("bhqd,bhkd->bhqk", qh, kh) / np.sqrt(np.float32(dk))
    s = np.where(mask, s, -np.inf)
    s = s - s.max(axis=-1, keepdims=True)
    e = np.exp(s)
    a = e / e.sum(axis=-1, keepdims=True)
    o = np.einsum("bhqk,bhkd->bhqd", a, vh)
    o = o.transpose(0, 2, 1, 3).reshape(Bn, Tn, H * dk)
    return (o @ Wo + bo).astype(np.float32)


def kernel(q, k, v, mask, Wq, bq, Wk, bk, Wv, bv, Wo, bo):
    global LAST_RESULTS
    q = np.asarray(q, np.float32)
    k = np.asarray(k, np.float32)
    v = np.asarray(v, np.float32)
    mask = np.asarray(mask, bool)
    Wq, bq = np.asarray(Wq, np.float32), np.asarray(bq, np.float32)
    Wk, bk = np.asarray(Wk, np.float32), np.asarray(bk, np.float32)
    Wv, bv = np.asarray(Wv, np.float32), np.asarray(bv, np.float32)
    Wo, bo = np.asarray(Wo, np.float32), np.asarray(bo, np.float32)

    if not mask.all():
        return _numpy_reference(q, k, v, mask, Wq, bq, Wk, bk, Wv, bv, Wo, bo)

    nc = _build_program()

    # host-side sharding
    xT = {}
    for b in range(B):
        xT[b] = tuple(np.ascontiguousarray(x[b].T) for x in (q, k, v))

    def w_chunks(W, g):
        # (1024, 256) head-group slice -> [128, 8*256] chunk-major layout
        Wg = W[:, g * GD:(g + 1) * GD]
        return np.ascontiguousarray(
            Wg.reshape(NF, 128, GD).transpose(1, 0, 2).reshape(128, NF * GD))

    in_maps = []
    for c in range(NCORES):
        b, g = divmod(c, GH)
        xq_t, xk_t, xv_t = xT[b]
        in_maps.append({
            "xqT": xq_t, "xkT": xk_t, "xvT": xv_t,
            "wq": w_chunks(Wq, g), "wk": w_chunks(Wk, g),
            "wv": w_chunks(Wv, g),
            "wo": np.ascontiguousarray(
                Wo[g * GD:(g + 1) * GD, :]).reshape(GH, DK, D),
            "bqv": np.ascontiguousarray(
                bq[g * GD:(g + 1) * GD].reshape(2, 128).T),
        })

    LAST_RESULTS = run_bass_kernel_spmd(
        nc, in_maps, list(range(NCORES)),
        trace=bool(os.environ.get("KERNEL_TRACE")))
    res = LAST_RESULTS.results

    const_row = (bv @ Wo + bo).astype(np.float32)  # attn rows sum to 1
    full = np.empty((B, T, D), np.float32)
    for b in range(B):
        acc = res[b * GH]["out"].astype(np.float32)
        for g in range(1, GH):
            acc = acc + res[b * GH + g]["out"]
        full[b] = acc + const_row
    return full
